# revision 1
# baseline (speedup 1.0000x reference)
"""Bass/Trainium2 kernel for nn_ChannelAttention (sparse_attention).

Math: per (batch b, 32-channel block n), q/k/v are per-channel affine maps of
x rows: q_d = A_d*x_d + B_d etc.  Hence q.k^T, the l2 norms, and attn@v are all
functions of the per-block channel Gram matrix G = X X^T and row sums S = X@1.
The whole module collapses to out[b] = BlockDiag(M_n) @ x[b] + beta, where the
M_n are 32x32 matrices derived from G,S via 16 tiny softmaxes (done on host).

Phase 1 (device, sharded over pixels): per-core partial [G | S] via PE
  transpose-by-identity + Gram matmul accumulation in PSUM.
Host: reduce partials, tiny softmax math -> M^T (block-diagonal 128x128), beta.
Phase 2 (device, sharded over pixels): out = M @ x + beta.
"""

import numpy as np

import concourse.bacc as bacc
import concourse.mybir as mybir
import concourse.tile as tile
import concourse.bass_utils as bass_utils
from concourse.masks import make_identity

B, C, H, W = 2, 128, 256, 256
HW = H * W
NCORES = 8
SH = HW // NCORES  # 8192 pixels per core
E = 2
NCH = 4
HEADS = NCH * E
D = C // NCH  # 32
EPS = 1e-12
F32 = mybir.dt.float32

CH = 2048  # dma chunk columns

_cache = {}


def _build_phase1():
    nc = bacc.Bacc("TRN2", target_bir_lowering=False, debug=False, num_devices=NCORES)
    x = nc.dram_tensor("x", [B, C, SH], F32, kind="ExternalInput").ap()
    gs = nc.dram_tensor("gs", [B, C, 129], F32, kind="ExternalOutput").ap()
    with tile.TileContext(nc) as tc:
        with (
            tc.tile_pool(name="const", bufs=1) as constp,
            tc.tile_pool(name="xin", bufs=3) as xinp,
            tc.tile_pool(name="xtps", bufs=4, space="PSUM") as xtpsp,
            tc.tile_pool(name="xtsb", bufs=4) as xtsbp,
            tc.tile_pool(name="gram", bufs=2, space="PSUM") as gramp,
            tc.tile_pool(name="gout", bufs=2) as goutp,
        ):
            ident = constp.tile([128, 128], F32)
            make_identity(nc, ident)
            nchunks = SH // 128  # 64
            for b in range(B):
                gram = gramp.tile([128, 132], F32, tag="gram")
                j = 0
                for jc in range(SH // CH):  # 4
                    x_t = xinp.tile([128, CH], F32, tag="xin")
                    nc.sync.dma_start(out=x_t, in_=x[b, :, jc * CH:(jc + 1) * CH])
                    for k in range(CH // 128):  # 16
                        src = x_t[:, k * 128:(k + 1) * 128]
                        xt_ps = xtpsp.tile([128, 128], F32, tag="xtps")
                        # xt = src.T  (PE transpose via identity moving operand)
                        nc.tensor.matmul(xt_ps, lhsT=src, rhs=ident,
                                         start=True, stop=True)
                        xt_sb = xtsbp.tile([128, 132], F32, tag="xtsb")
                        nc.scalar.copy(xt_sb[:, 0:128], xt_ps)
                        nc.vector.memset(xt_sb[:, 128:129], 1.0)
                        # [G | S] += xt.T @ [xt | 1]
                        nc.tensor.matmul(gram[:, 0:129],
                                         lhsT=xt_sb[:, 0:128],
                                         rhs=xt_sb[:, 0:129],
                                         start=(j == 0), stop=(j == nchunks - 1))
                        j += 1
                go = goutp.tile([128, 129], F32, tag="gout")
                nc.vector.tensor_copy(go, gram[:, 0:129])
                nc.sync.dma_start(out=gs[b], in_=go)
    nc.compile()
    return nc


def _build_phase2():
    nc = bacc.Bacc("TRN2", target_bir_lowering=False, debug=False, num_devices=NCORES)
    x = nc.dram_tensor("x", [B, C, SH], F32, kind="ExternalInput").ap()
    mt = nc.dram_tensor("mt", [B, C, C], F32, kind="ExternalInput").ap()
    beta = nc.dram_tensor("beta", [B, C, 1], F32, kind="ExternalInput").ap()
    out = nc.dram_tensor("out", [B, C, SH], F32, kind="ExternalOutput").ap()
    with tile.TileContext(nc) as tc:
        with (
            tc.tile_pool(name="wts", bufs=1) as wp,
            tc.tile_pool(name="xin", bufs=3) as xinp,
            tc.tile_pool(name="ps", bufs=4, space="PSUM") as psp,
            tc.tile_pool(name="osb", bufs=3) as osbp,
        ):
            for b in range(B):
                mt_sb = wp.tile([128, 128], F32, tag=f"mt{b}")
                nc.sync.dma_start(out=mt_sb, in_=mt[b])
                beta_sb = wp.tile([128, 1], F32, tag=f"beta{b}")
                nc.sync.dma_start(out=beta_sb, in_=beta[b])
                for jc in range(SH // CH):  # 4
                    x_t = xinp.tile([128, CH], F32, tag="xin")
                    nc.sync.dma_start(out=x_t, in_=x[b, :, jc * CH:(jc + 1) * CH])
                    o_sb = osbp.tile([128, CH], F32, tag="osb")
                    for k in range(CH // 512):  # 4
                        ps = psp.tile([128, 512], F32, tag="ps")
                        nc.tensor.matmul(ps, lhsT=mt_sb,
                                         rhs=x_t[:, k * 512:(k + 1) * 512],
                                         start=True, stop=True)
                        nc.vector.tensor_scalar_add(
                            o_sb[:, k * 512:(k + 1) * 512], in0=ps,
                            scalar1=beta_sb)
                    nc.sync.dma_start(out=out[b, :, jc * CH:(jc + 1) * CH],
                                      in_=o_sb)
    nc.compile()
    return nc


def _softmax(a, axis=-1):
    m = np.max(a, axis=axis, keepdims=True)
    ex = np.exp(a - m)
    return ex / np.sum(ex, axis=axis, keepdims=True)


def _host_mbeta(G, S, w_qkv, b_qkv, w_fus, b_fus, t):
    """From per-batch Gram G [B,128,128] and row sums S [B,128], build
    M^T [B,128,128] (block-diagonal) and beta [B,128]."""
    N = float(HW)
    t = t.reshape(HEADS)
    M = np.zeros((B, C, C), dtype=np.float64)
    beta = np.zeros((B, C), dtype=np.float64)
    for b in range(B):
        for n in range(NCH):
            sl = slice(n * D, (n + 1) * D)
            Gb = G[b][sl, sl]
            dG = np.diag(Gb)
            Sb = S[b][sl]
            Mn = np.zeros((D, D), dtype=np.float64)
            bn = np.zeros(D, dtype=np.float64)
            for e in range(E):
                h = e * NCH + n
                A = w_qkv[sl, e]
                Bv = b_qkv[sl, e]
                Cv = w_qkv[sl, E + e]
                Dv = b_qkv[sl, E + e]
                Vv = w_qkv[sl, 2 * E + e]
                Uv = b_qkv[sl, 2 * E + e]
                qk = ((A[:, None] * Cv[None, :]) * Gb
                      + (A * Sb)[:, None] * Dv[None, :]
                      + Bv[:, None] * (Cv * Sb)[None, :]
                      + N * (Bv[:, None] * Dv[None, :]))
                nq = np.sqrt(np.maximum(A * A * dG + 2 * A * Bv * Sb + Bv * Bv * N, 0.0))
                nk = np.sqrt(np.maximum(Cv * Cv * dG + 2 * Cv * Dv * Sb + Dv * Dv * N, 0.0))
                L = t[h] * qk / np.maximum(nq, EPS)[:, None] / np.maximum(nk, EPS)[None, :]
                P = _softmax(L, axis=-1)
                Mn += w_fus[sl, e][:, None] * (P * Vv[None, :])
                bn += w_fus[sl, e] * (P @ Uv)
            bn += b_fus[sl]
            M[b][sl, sl] = Mn
            beta[b][sl] = bn
    mtr = np.ascontiguousarray(M.transpose(0, 2, 1)).astype(np.float32)
    return mtr, beta.astype(np.float32).reshape(B, C, 1)


def kernel(x, w_qkv, b_qkv, w_fus, b_fus, t, _profile=None):
    x = np.asarray(x, dtype=np.float32)
    w_qkv = np.asarray(w_qkv, dtype=np.float64)
    b_qkv = np.asarray(b_qkv, dtype=np.float64)
    w_fus = np.asarray(w_fus, dtype=np.float64)
    b_fus = np.asarray(b_fus, dtype=np.float64)
    t = np.asarray(t, dtype=np.float64)

    if "p1" not in _cache:
        _cache["p1"] = _build_phase1()
    if "p2" not in _cache:
        _cache["p2"] = _build_phase2()

    xf = x.reshape(B, C, HW)
    shards = [np.ascontiguousarray(xf[:, :, i * SH:(i + 1) * SH])
              for i in range(NCORES)]

    kw1 = dict(_profile or {})
    res1 = bass_utils.run_bass_kernel_spmd(
        _cache["p1"], [{"x": s} for s in shards],
        core_ids=list(range(NCORES)), **kw1)
    gs = np.sum([r["gs"].astype(np.float64) for r in res1.results], axis=0)
    G = gs[:, :, 0:128]
    S = gs[:, :, 128]

    mtr, beta = _host_mbeta(G, S, w_qkv, b_qkv, w_fus, b_fus, t)

    kw2 = dict(_profile or {})
    res2 = bass_utils.run_bass_kernel_spmd(
        _cache["p2"],
        [{"x": s, "mt": mtr, "beta": beta} for s in shards],
        core_ids=list(range(NCORES)), **kw2)
    out = np.concatenate([r["out"] for r in res2.results], axis=2)
    if _profile is not None:
        _profile["results"] = (res1, res2)
    return out.reshape(B, C, H, W)
